# revision 31
# baseline (speedup 1.0000x reference)
"""Trainium2 Bass kernel for nn_DerivNet (MLP 8->1024->1024->1 forward + analytic Jacobian).

Strategy
--------
Data-parallel over the batch axis: 8192 samples -> 1024 per NeuronCore, weights
replicated.  Because the network output is scalar (DO=1), the reference's
per-sample Jacobian chain (einsum over [h,h]@[h,dx] per sample, ~137 GFLOP) is
collapsed from the output side into a VJP:

    z1 = tanh(x @ W1^T + b1)
    z2 = tanh(z1 @ W2^T + b2)
    y  = z2 @ W3^T + b3
    d2w = (1 - z2^2) * W3[0]          # [nx, h]
    t   = d2w @ W2                    # [nx, h]   <- the only other big matmul
    s   = t * (1 - z1^2)              # [nx, h]
    dydx[k, n, 0] = (s @ W1)[n, k]    # -> [dx, nx, 1]

Total ~4.3 GFLOP/core, dominated by two 1024^3 matmuls.

On-chip layout is feature-major ([h, n] with h on partitions), so biases and
the W3 scaling are per-partition scalars.  All matmul operands use fp32r
(full-rate fp32 on the PE; exact when inputs are pre-rounded to the tf32
grid, which the host does for weights/x).  Host pre-transposes W2/W1 so the
device never transposes anything.

Schedule notes: one unified 8-bank PSUM pool so the PE never waits on bank
rotation; layer-1 packed 4-wide into PE row groups (K=8 -> tile_position);
forward chunk 0 interleaved with layer-1; D1 precomputed during the forward
phase; y/d2w blocks trail their producer by one PE phase; dydx matmuls
emitted right after each backward group's drains; W2/W2T DMAs split into
half-tiles ordered by first use, with W2 (backward-only) gated behind W2T so
the two DMA queues don't split HBM bandwidth on the critical path; ACT table
pre-warmed during the engine preamble; two throwaway matmuls warm the PE
(HAM un-throttle) while the input DMAs are in flight; the last backward
groups run per-bank chains so their s-mul drains and the dydx matmuls
pipeline instead of gating the tail.  Measured ~89 us on trn2 (NTFF):
~76 us gap-free PE stream from t=7.5us + ~1.5 us tail + ~4 us teardown,
vs ~65 us of pure matmul streaming at the fp32r roofline.
"""

import sys

sys.path.insert(0, "/opt/trn_rl_repo")

import numpy as np

import concourse.mybir as mybir
import concourse.tile as tile
from concourse import bacc
from concourse.bass_utils import run_bass_kernel_spmd

NX, DX, H, DO = 8192, 8, 1024, 1
NCORES = 8
NXL = NX // NCORES  # samples per core
P = 128
HT = H // P  # 8 feature tiles of 128
CH = 512  # free-dim chunk (one PSUM bank of fp32)
NCH = NXL // CH  # 2 chunks

F32 = mybir.dt.float32
F32R = mybir.dt.float32r
AF = mybir.ActivationFunctionType
OP = mybir.AluOpType


def _round_tf32(x: np.ndarray) -> np.ndarray:
    """Round fp32 to the tf32 grid (10-bit mantissa, RNE) required by fp32r."""
    u = np.ascontiguousarray(x, dtype=np.float32).view(np.uint32)
    r = ((u.astype(np.uint64) + 0x1000 + ((u >> 13) & 1)) & 0xFFFFE000).astype(np.uint32)
    return r.view(np.float32)


def _emit(nc, t):
    with tile.TileContext(nc) as tc:
        with (
            tc.tile_pool(name="consts", bufs=1) as consts,
            tc.tile_pool(name="outs", bufs=1) as outs,
            tc.tile_pool(name="w2p", bufs=1) as w2p,
            tc.tile_pool(name="w2tp", bufs=1) as w2tp,
            tc.tile_pool(name="z1p", bufs=1) as z1p,
            tc.tile_pool(name="z2p", bufs=1) as z2p,
            tc.tile_pool(name="d1p", bufs=1) as d1p,
            tc.tile_pool(name="tmp", bufs=6) as tmp,
            tc.tile_pool(name="acc", bufs=8, space="PSUM") as accp,
        ):
            def psum(nm, rows=P):
                return accp.tile([rows, CH], F32, tag="acc", name=nm)

            # ---- constants / small inputs -------------------------------
            # xT is host-replicated to partition offsets 0/32/64/96 and w1t is
            # packed per row-group so layer-1 runs as 4 concurrent K=8 matmuls
            # via tile_position.
            xT_sb = consts.tile([P, NXL], F32R)
            w1t_sb = consts.tile([P, 2 * P], F32R)
            w1r_sb = consts.tile([P, HT * DX], F32R)
            b1r_sb = consts.tile([P, HT], F32)
            b2r_sb = consts.tile([P, HT], F32)
            w3mm_sb = consts.tile([P, HT], F32R)
            w3p_sb = consts.tile([P, HT], F32)
            w3n_sb = consts.tile([P, HT], F32)
            b3_sb = consts.tile([1, 1], F32)
            y_sb = outs.tile([DO, NXL], F32)
            dydx_sb = outs.tile([DX, NXL], F32)
            w2t_sb = w2tp.tile([P, HT, H], F32R, tag="w2t_s")
            w2_sb = w2p.tile([P, HT, H], F32R)

            # Warm the ACT table (Tanh set) with a no-dependency op so the
            # ~1.3us ACT_TABLE_LOAD runs during the DMA window.
            warm = consts.tile([1, 1], F32, name="warm")
            warm2 = consts.tile([1, 1], F32, name="warm2")
            nc.vector.memset(warm[:], 0.0)
            nc.scalar.activation(warm2[:], warm[:], AF.Tanh)
            # Warm the PE (HAM un-throttle needs ~3.4us of CONTINUOUS activity)
            # with throwaway matmuls on a zeroed tile: one before layer-1's
            # first wave (fills the input-DMA wait), two right after it (they
            # complete the HAM window while the tanh chain runs, without
            # delaying its start).
            wsrc = consts.tile([P, CH], F32, name="wsrc")
            nc.vector.memset(wsrc[:], 0.0)

            def warm_mm(nm):
                pw = accp.tile([P, CH], F32, tag="acc", name=nm)
                nc.tensor.matmul(pw[:], wsrc[:, 0:P], wsrc[:])

            warm_mm("warmps0")

            # DMA issue order matters: each queue is serial (~0.7us per DMA).
            # Sync queue: the layer-1/forward critical path (xT, w1t, b1r,
            # then W2T halves, group-0 halves first).  GpSimd queue: the
            # drain-time constants, then (after W2T has landed - enforced by
            # a tiny blocker DMA reading w2t_sb, so the two queues do not
            # split HBM bandwidth while the forward pass waits on W2T) the
            # backward weights W2.
            for name, sb in [("w1t", w1t_sb), ("xT", xT_sb)]:
                nc.sync.dma_start(out=sb[:], in_=t[name].ap())
            nc.gpsimd.dma_start(out=b1r_sb[:], in_=t["b1r"].ap())
            h0 = slice(0, CH)
            h1 = slice(CH, H)
            for k in range(HT):
                nc.sync.dma_start(
                    out=w2t_sb[:, k, h0], in_=t["w2t"].ap()[k * P:(k + 1) * P, h0]
                )
            for name, sb in [
                ("b2r", b2r_sb), ("w3mm", w3mm_sb), ("w3p", w3p_sb),
                ("w3n", w3n_sb), ("b3", b3_sb), ("w1r", w1r_sb),
            ]:
                nc.gpsimd.dma_start(out=sb[:], in_=t[name].ap())
            for k in range(HT):
                nc.gpsimd.dma_start(
                    out=w2t_sb[:, k, h1], in_=t["w2t"].ap()[k * P:(k + 1) * P, h1]
                )
            # Gate W2 (backward-only) behind BOTH W2T streams so it doesn't
            # steal HBM bandwidth from the forward critical path: the blocker
            # reads the last element of the sync-queue half; the gpsimd half
            # is ordered by the queue itself.
            w2gate = consts.tile([1, 1], F32R, name="w2gate")
            nc.gpsimd.dma_start(out=w2gate[:], in_=w2t_sb[0:1, HT - 1, CH - 1:CH])
            for half in range(2):
                hs = slice(half * CH, (half + 1) * CH)
                for k in range(HT):
                    nc.gpsimd.dma_start(
                        out=w2_sb[:, k, hs], in_=t["w2"].ap()[k * P:(k + 1) * P, hs]
                    )

            z1_sb = z1p.tile([P, HT, NXL], F32R)
            z2_sb = z2p.tile([P, HT, NXL], F32R)  # overwritten by d2w after y
            d1_sb = d1p.tile([P, HT, NXL], F32)

            def layer1(c, half):
                # 4 concurrent K=8 matmuls in row groups 0/32/64/96
                pss = [psum(f"l1_{c}{half}{g}") for g in range(4)]
                for g in range(4):
                    nc.tensor.matmul(
                        pss[g][:],
                        w1t_sb[32 * g:32 * g + DX, half * P:(half + 1) * P],
                        xT_sb[32 * g:32 * g + DX, c * CH:(c + 1) * CH],
                        tile_position=(32 * g, 0),
                    )
                for g in range(4):
                    hi = half * 4 + g
                    nc.scalar.activation(
                        z1_sb[:, hi, c * CH:(c + 1) * CH], pss[g][:], AF.Tanh,
                        bias=b1r_sb[:, hi:hi + 1],
                    )

            def fwd_group(c, g):
                cs = slice(c * CH, (c + 1) * CH)
                pss = [psum(f"f{c}{g}{u}") for u in range(4)]
                for ki in range(HT):
                    for u, mi in enumerate(range(g * 4, g * 4 + 4)):
                        nc.tensor.matmul(
                            pss[u][:],
                            w2t_sb[:, ki, mi * P:(mi + 1) * P],
                            z1_sb[:, ki, cs],
                            start=(ki == 0),
                            stop=(ki == HT - 1),
                        )
                for u, mi in enumerate(range(g * 4, g * 4 + 4)):
                    nc.scalar.activation(
                        z2_sb[:, mi, cs], pss[u][:], AF.Tanh,
                        bias=b2r_sb[:, mi:mi + 1],
                    )

            def d1_block(c):
                # d1 = 1 - z1^2 entirely on DVE (ACT is busy with tanh)
                cs = slice(c * CH, (c + 1) * CH)
                for hi in range(HT):
                    sq = tmp.tile([P, CH], F32, tag="tmp", name=f"zsq{c}{hi}")
                    nc.vector.tensor_mul(sq[:], z1_sb[:, hi, cs], z1_sb[:, hi, cs])
                    nc.vector.tensor_scalar(
                        out=d1_sb[:, hi, cs], in0=sq[:], scalar1=-1.0, scalar2=1.0,
                        op0=OP.mult, op1=OP.add,
                    )

            def y_block(c):
                cs = slice(c * CH, (c + 1) * CH)
                psy = psum(f"y{c}", rows=DO)
                for ki in range(HT):
                    nc.tensor.matmul(
                        psy[:], w3mm_sb[:, ki:ki + 1], z2_sb[:, ki, cs],
                        start=(ki == 0), stop=(ki == HT - 1),
                    )
                nc.vector.tensor_scalar_add(out=y_sb[:, cs], in0=psy[:], scalar1=b3_sb[:, 0:1])

            def d2w_block(c):
                # d2w = w3 * (1 - z2^2), overwriting z2 in place (Square on ACT)
                cs = slice(c * CH, (c + 1) * CH)
                for mi in range(HT):
                    sq = tmp.tile([P, CH], F32, tag="tmp", name=f"z2sq{c}{mi}")
                    nc.scalar.activation(sq[:], z2_sb[:, mi, cs], AF.Square)
                    nc.vector.tensor_scalar(
                        out=z2_sb[:, mi, cs], in0=sq[:],
                        scalar1=w3n_sb[:, mi:mi + 1], scalar2=w3p_sb[:, mi:mi + 1],
                        op0=OP.mult, op1=OP.add,
                    )

            def bwd_group(c, g, chain=False):
                cs = slice(c * CH, (c + 1) * CH)
                pss = [psum(f"b{c}{g}{u}") for u in range(4)]
                if chain:
                    # per-bank chains: each bank finishes (and its s-mul
                    # drains) as early as possible instead of all four gating
                    # the dydx tail together
                    for u, mi in enumerate(range(g * 4, g * 4 + 4)):
                        for kj in range(HT):
                            nc.tensor.matmul(
                                pss[u][:],
                                w2_sb[:, kj, mi * P:(mi + 1) * P],
                                z2_sb[:, kj, cs],
                                start=(kj == 0),
                                stop=(kj == HT - 1),
                            )
                        nc.vector.tensor_mul(
                            s_sb[:, mi, cs], pss[u][:], d1_sb[:, mi, cs]
                        )
                    return
                for kj in range(HT):
                    for u, mi in enumerate(range(g * 4, g * 4 + 4)):
                        nc.tensor.matmul(
                            pss[u][:],
                            w2_sb[:, kj, mi * P:(mi + 1) * P],
                            z2_sb[:, kj, cs],
                            start=(kj == 0),
                            stop=(kj == HT - 1),
                        )
                for u, mi in enumerate(range(g * 4, g * 4 + 4)):
                    nc.vector.tensor_mul(
                        s_sb[:, mi, cs], pss[u][:], d1_sb[:, mi, cs]
                    )

            def dydx_block(c):
                cs = slice(c * CH, (c + 1) * CH)
                psd = psum(f"dydx{c}", rows=DX)
                for ki in range(HT):
                    nc.tensor.matmul(
                        psd[:], w1r_sb[:, ki * DX:(ki + 1) * DX], s_sb[:, ki, cs],
                        start=(ki == 0), stop=(ki == HT - 1),
                    )
                nc.vector.tensor_copy(dydx_sb[:, cs], psd[:])
                nc.sync.dma_start(out=t["dydx"].ap()[:, cs], in_=dydx_sb[:, cs])

            # ---- emission order (PE stream order) -----------------------
            # y/d2w blocks trail their producer by one PE phase so the PE
            # never waits on the ACT tanh/Square drain chain.
            layer1(0, 0)
            warm_mm("warmps1")
            warm_mm("warmps2")
            layer1(0, 1)
            layer1(1, 0)
            fwd_group(0, 0)   # starts early, paced by layer-1 tanh + W2T DMA
            layer1(1, 1)
            fwd_group(0, 1)
            d1_block(0)
            d1_block(1)
            fwd_group(1, 0)
            y_block(0)
            d2w_block(0)
            fwd_group(1, 1)

            s_sb = w2tp.tile([P, HT, NXL], F32R, tag="w2t_s")  # reuses w2t slot
            bwd_group(0, 0)
            y_block(1)
            d2w_block(1)
            nc.sync.dma_start(out=t["y"].ap(), in_=y_sb[:])
            bwd_group(0, 1)
            bwd_group(1, 0, chain=True)
            dydx_block(0)
            bwd_group(1, 1, chain=True)
            dydx_block(1)


def _build():
    nc = bacc.Bacc("TRN2", target_bir_lowering=False, debug=False, num_devices=NCORES)
    t = {}
    for name, shape, dt in [
        ("xT", [P, NXL], F32R),
        ("w1t", [P, 2 * P], F32R),
        ("w1r", [P, HT * DX], F32R),
        ("w2", [H, H], F32R),
        ("w2t", [H, H], F32R),
        ("b1r", [P, HT], F32),
        ("b2r", [P, HT], F32),
        ("w3mm", [P, HT], F32R),
        ("w3p", [P, HT], F32),
        ("w3n", [P, HT], F32),
        ("b3", [1, 1], F32),
    ]:
        t[name] = nc.dram_tensor(name, shape, dt, kind="ExternalInput")
    t["y"] = nc.dram_tensor("y", [DO, NXL], F32, kind="ExternalOutput")
    t["dydx"] = nc.dram_tensor("dydx", [DX, NXL], F32, kind="ExternalOutput")
    _emit(nc, t)
    nc.compile()
    return nc


_NC = None


def _get_nc():
    global _NC
    if _NC is None:
        _NC = _build()
    return _NC


def _prep_in_maps(inputs):
    x = np.asarray(inputs["x"], dtype=np.float32)
    W1 = np.asarray(inputs["W1"], dtype=np.float32)
    b1 = np.asarray(inputs["b1"], dtype=np.float32)
    W2 = np.asarray(inputs["W2"], dtype=np.float32)
    b2 = np.asarray(inputs["b2"], dtype=np.float32)
    W3 = np.asarray(inputs["W3"], dtype=np.float32)
    b3 = np.asarray(inputs["b3"], dtype=np.float32)

    # xT replicated to partition offsets 0/32/64/96 for 4-wide packed layer-1
    xT = _round_tf32(x.T)  # [8, 8192]
    xT_q = np.zeros((P, NX), dtype=np.float32)
    for g in range(4):
        xT_q[32 * g:32 * g + DX] = xT
    # w1t packed per row-group: block (g, half) holds W1T[:, (half*4+g)-th tile]
    w1t_q = np.zeros((P, 2 * P), dtype=np.float32)
    W1T_r = _round_tf32(W1.T)  # [8, 1024]
    for half in range(2):
        for g in range(4):
            hi = half * 4 + g
            w1t_q[32 * g:32 * g + DX, half * P:(half + 1) * P] = W1T_r[:, hi * P:(hi + 1) * P]
    # W1 row-tiles stacked along free dim: [128, 8*8], col block ki = W1[ki*128+p, :]
    w1r = _round_tf32(W1.reshape(HT, P, DX).transpose(1, 0, 2).reshape(P, HT * DX))
    w2 = _round_tf32(W2)
    w2t = _round_tf32(W2.T)
    b1r = np.ascontiguousarray(b1.reshape(HT, P).T)  # [128, 8]
    b2r = np.ascontiguousarray(b2.reshape(HT, P).T)
    w3r = np.ascontiguousarray(W3[0].reshape(HT, P).T)  # [128, 8]
    w3mm = _round_tf32(w3r)
    w3n = -w3r
    b3v = b3.reshape(1, 1)

    shared = {
        "w1t": w1t_q, "w1r": w1r, "w2": w2, "w2t": w2t,
        "b1r": b1r, "b2r": b2r, "w3mm": w3mm, "w3p": w3r, "w3n": w3n, "b3": b3v,
    }
    in_maps = []
    for c in range(NCORES):
        m = dict(shared)
        m["xT"] = np.ascontiguousarray(xT_q[:, c * NXL:(c + 1) * NXL])
        in_maps.append(m)
    return in_maps


def run(inputs, trace=False):
    nc = _get_nc()
    in_maps = _prep_in_maps(inputs)
    res = run_bass_kernel_spmd(nc, in_maps, list(range(NCORES)), trace=trace)
    y = np.concatenate([res.results[c]["y"][0] for c in range(NCORES)])[:, None]
    dydx = np.concatenate([res.results[c]["dydx"] for c in range(NCORES)], axis=1)[:, :, None]
    out = (y.astype(np.float32), dydx.astype(np.float32))
    return out, res.exec_time_ns


def kernel(**inputs):
    out, _ = run(inputs, trace=False)
    return out


# revision 32
# speedup vs baseline: 1.0192x; 1.0192x over previous
"""Trainium2 Bass kernel for nn_DerivNet (MLP 8->1024->1024->1 forward + analytic Jacobian).

Strategy
--------
Data-parallel over the batch axis: 8192 samples -> 1024 per NeuronCore, weights
replicated.  Because the network output is scalar (DO=1), the reference's
per-sample Jacobian chain (einsum over [h,h]@[h,dx] per sample, ~137 GFLOP) is
collapsed from the output side into a VJP:

    z1 = tanh(x @ W1^T + b1)
    z2 = tanh(z1 @ W2^T + b2)
    y  = z2 @ W3^T + b3
    d2w = (1 - z2^2) * W3[0]          # [nx, h]
    t   = d2w @ W2                    # [nx, h]   <- the only other big matmul
    s   = t * (1 - z1^2)              # [nx, h]
    dydx[k, n, 0] = (s @ W1)[n, k]    # -> [dx, nx, 1]

Total ~4.3 GFLOP/core, dominated by two 1024^3 matmuls.

On-chip layout is feature-major ([h, n] with h on partitions), so biases and
the W3 scaling are per-partition scalars.  All matmul operands use fp32r
(full-rate fp32 on the PE; exact when inputs are pre-rounded to the tf32
grid, which the host does for weights/x).  Host pre-transposes W2/W1 so the
device never transposes anything.

Schedule notes: one unified 8-bank PSUM pool so the PE never waits on bank
rotation; layer-1 packed 4-wide into PE row groups (K=8 -> tile_position);
forward chunk 0 interleaved with layer-1; D1 precomputed during the forward
phase; y/d2w blocks trail their producer by one PE phase; dydx matmuls
emitted right after each backward group's drains; W2/W2T DMAs split into
half-tiles ordered by first use, with W2 (backward-only) gated behind W2T so
the two DMA queues don't split HBM bandwidth on the critical path; ACT table
pre-warmed during the engine preamble; two throwaway matmuls warm the PE
(HAM un-throttle) while the input DMAs are in flight; the last backward
groups run per-bank chains so their s-mul drains and the dydx matmuls
pipeline instead of gating the tail.  Measured ~89 us on trn2 (NTFF):
~76 us gap-free PE stream from t=7.5us + ~1.5 us tail + ~4 us teardown,
vs ~65 us of pure matmul streaming at the fp32r roofline.
"""

import sys

sys.path.insert(0, "/opt/trn_rl_repo")

import numpy as np

import concourse.mybir as mybir
import concourse.tile as tile
from concourse import bacc
from concourse.bass_utils import run_bass_kernel_spmd

NX, DX, H, DO = 8192, 8, 1024, 1
NCORES = 8
NXL = NX // NCORES  # samples per core
P = 128
HT = H // P  # 8 feature tiles of 128
CH = 512  # free-dim chunk (one PSUM bank of fp32)
NCH = NXL // CH  # 2 chunks

F32 = mybir.dt.float32
F32R = mybir.dt.float32r
AF = mybir.ActivationFunctionType
OP = mybir.AluOpType


def _round_tf32(x: np.ndarray) -> np.ndarray:
    """Round fp32 to the tf32 grid (10-bit mantissa, RNE) required by fp32r."""
    u = np.ascontiguousarray(x, dtype=np.float32).view(np.uint32)
    r = ((u.astype(np.uint64) + 0x1000 + ((u >> 13) & 1)) & 0xFFFFE000).astype(np.uint32)
    return r.view(np.float32)


def _emit(nc, t):
    with tile.TileContext(nc) as tc:
        with (
            tc.tile_pool(name="consts", bufs=1) as consts,
            tc.tile_pool(name="outs", bufs=1) as outs,
            tc.tile_pool(name="w2p", bufs=1) as w2p,
            tc.tile_pool(name="w2tp", bufs=1) as w2tp,
            tc.tile_pool(name="z1p", bufs=1) as z1p,
            tc.tile_pool(name="z2p", bufs=1) as z2p,
            tc.tile_pool(name="d1p", bufs=1) as d1p,
            tc.tile_pool(name="tmp", bufs=6) as tmp,
            tc.tile_pool(name="acc", bufs=8, space="PSUM") as accp,
        ):
            def psum(nm, rows=P):
                return accp.tile([rows, CH], F32, tag="acc", name=nm)

            # ---- constants / small inputs -------------------------------
            # xT is host-replicated to partition offsets 0/32/64/96 and w1t is
            # packed per row-group so layer-1 runs as 4 concurrent K=8 matmuls
            # via tile_position.
            xT_sb = consts.tile([P, NXL], F32R)
            w1t_sb = consts.tile([P, 2 * P], F32R)
            w1r_sb = consts.tile([P, HT * DX], F32R)
            b1r_sb = consts.tile([P, HT], F32)
            b2r_sb = consts.tile([P, HT], F32)
            w3mm_sb = consts.tile([P, HT], F32R)
            w3p_sb = consts.tile([P, HT], F32)
            w3n_sb = consts.tile([P, HT], F32)
            b3_sb = consts.tile([1, 1], F32)
            y_sb = outs.tile([DO, NXL], F32)
            dydx_sb = outs.tile([DX, NXL], F32)
            w2t_sb = w2tp.tile([P, HT, H], F32R, tag="w2t_s")
            w2_sb = w2p.tile([P, HT, H], F32R)

            # Warm the ACT table (Tanh set) with a no-dependency op so the
            # ~1.3us ACT_TABLE_LOAD runs during the DMA window.
            warm = consts.tile([1, 1], F32, name="warm")
            warm2 = consts.tile([1, 1], F32, name="warm2")
            nc.vector.memset(warm[:], 0.0)
            nc.scalar.activation(warm2[:], warm[:], AF.Tanh)
            # Warm the PE (HAM un-throttle) with two throwaway matmuls on a
            # zeroed tile while the input DMAs are still in flight.  Count and
            # size are HW-tuned: 1 or 3+ warm matmuls, shorter ones, and
            # pre-warming around layer-1 all measured worse.
            wsrc = consts.tile([P, CH], F32, name="wsrc")
            nc.vector.memset(wsrc[:], 0.0)
            pw = psum("warmps")
            for r in range(2):
                nc.tensor.matmul(pw[:], wsrc[:, 0:P], wsrc[:], start=(r == 0), stop=(r == 1))

            # DMA issue order matters: each queue is serial (~0.7us per DMA).
            # Sync queue: the layer-1/forward critical path (xT, w1t, b1r,
            # then W2T halves, group-0 halves first).  GpSimd queue: the
            # drain-time constants, then (after W2T has landed - enforced by
            # a tiny blocker DMA reading w2t_sb, so the two queues do not
            # split HBM bandwidth while the forward pass waits on W2T) the
            # backward weights W2.
            for name, sb in [("w1t", w1t_sb), ("xT", xT_sb)]:
                nc.sync.dma_start(out=sb[:], in_=t[name].ap())
            nc.gpsimd.dma_start(out=b1r_sb[:], in_=t["b1r"].ap())
            h0 = slice(0, CH)
            h1 = slice(CH, H)
            for k in range(HT):
                nc.sync.dma_start(
                    out=w2t_sb[:, k, h0], in_=t["w2t"].ap()[k * P:(k + 1) * P, h0]
                )
            for name, sb in [
                ("b2r", b2r_sb), ("w3mm", w3mm_sb), ("w3p", w3p_sb),
                ("w3n", w3n_sb), ("b3", b3_sb), ("w1r", w1r_sb),
            ]:
                nc.gpsimd.dma_start(out=sb[:], in_=t[name].ap())
            for k in range(HT):
                nc.gpsimd.dma_start(
                    out=w2t_sb[:, k, h1], in_=t["w2t"].ap()[k * P:(k + 1) * P, h1]
                )
            # Gate W2 (backward-only) behind BOTH W2T streams so it doesn't
            # steal HBM bandwidth from the forward critical path: the blocker
            # reads the last element of the sync-queue half; the gpsimd half
            # is ordered by the queue itself.
            w2gate = consts.tile([1, 1], F32R, name="w2gate")
            nc.gpsimd.dma_start(out=w2gate[:], in_=w2t_sb[0:1, HT - 1, CH - 1:CH])
            for half in range(2):
                hs = slice(half * CH, (half + 1) * CH)
                for k in range(HT):
                    nc.gpsimd.dma_start(
                        out=w2_sb[:, k, hs], in_=t["w2"].ap()[k * P:(k + 1) * P, hs]
                    )

            z1_sb = z1p.tile([P, HT, NXL], F32R)
            z2_sb = z2p.tile([P, HT, NXL], F32R)  # overwritten by d2w after y
            d1_sb = d1p.tile([P, HT, NXL], F32)

            def layer1(c, half):
                # 4 concurrent K=8 matmuls in row groups 0/32/64/96
                pss = [psum(f"l1_{c}{half}{g}") for g in range(4)]
                for g in range(4):
                    nc.tensor.matmul(
                        pss[g][:],
                        w1t_sb[32 * g:32 * g + DX, half * P:(half + 1) * P],
                        xT_sb[32 * g:32 * g + DX, c * CH:(c + 1) * CH],
                        tile_position=(32 * g, 0),
                    )
                for g in range(4):
                    hi = half * 4 + g
                    nc.scalar.activation(
                        z1_sb[:, hi, c * CH:(c + 1) * CH], pss[g][:], AF.Tanh,
                        bias=b1r_sb[:, hi:hi + 1],
                    )

            def fwd_group(c, g):
                cs = slice(c * CH, (c + 1) * CH)
                pss = [psum(f"f{c}{g}{u}") for u in range(4)]
                for ki in range(HT):
                    for u, mi in enumerate(range(g * 4, g * 4 + 4)):
                        nc.tensor.matmul(
                            pss[u][:],
                            w2t_sb[:, ki, mi * P:(mi + 1) * P],
                            z1_sb[:, ki, cs],
                            start=(ki == 0),
                            stop=(ki == HT - 1),
                        )
                for u, mi in enumerate(range(g * 4, g * 4 + 4)):
                    nc.scalar.activation(
                        z2_sb[:, mi, cs], pss[u][:], AF.Tanh,
                        bias=b2r_sb[:, mi:mi + 1],
                    )

            def d1_block(c):
                # d1 = 1 - z1^2 entirely on DVE (ACT is busy with tanh)
                cs = slice(c * CH, (c + 1) * CH)
                for hi in range(HT):
                    sq = tmp.tile([P, CH], F32, tag="tmp", name=f"zsq{c}{hi}")
                    nc.vector.tensor_mul(sq[:], z1_sb[:, hi, cs], z1_sb[:, hi, cs])
                    nc.vector.tensor_scalar(
                        out=d1_sb[:, hi, cs], in0=sq[:], scalar1=-1.0, scalar2=1.0,
                        op0=OP.mult, op1=OP.add,
                    )

            def y_block(c):
                cs = slice(c * CH, (c + 1) * CH)
                psy = psum(f"y{c}", rows=DO)
                for ki in range(HT):
                    nc.tensor.matmul(
                        psy[:], w3mm_sb[:, ki:ki + 1], z2_sb[:, ki, cs],
                        start=(ki == 0), stop=(ki == HT - 1),
                    )
                nc.vector.tensor_scalar_add(out=y_sb[:, cs], in0=psy[:], scalar1=b3_sb[:, 0:1])

            def d2w_block(c):
                # d2w = w3 * (1 - z2^2), overwriting z2 in place (Square on ACT)
                cs = slice(c * CH, (c + 1) * CH)
                for mi in range(HT):
                    sq = tmp.tile([P, CH], F32, tag="tmp", name=f"z2sq{c}{mi}")
                    nc.scalar.activation(sq[:], z2_sb[:, mi, cs], AF.Square)
                    nc.vector.tensor_scalar(
                        out=z2_sb[:, mi, cs], in0=sq[:],
                        scalar1=w3n_sb[:, mi:mi + 1], scalar2=w3p_sb[:, mi:mi + 1],
                        op0=OP.mult, op1=OP.add,
                    )

            def bwd_group(c, g, chain=False):
                cs = slice(c * CH, (c + 1) * CH)
                pss = [psum(f"b{c}{g}{u}") for u in range(4)]
                if chain:
                    # per-bank chains: each bank finishes (and its s-mul
                    # drains) as early as possible instead of all four gating
                    # the dydx tail together
                    for u, mi in enumerate(range(g * 4, g * 4 + 4)):
                        for kj in range(HT):
                            nc.tensor.matmul(
                                pss[u][:],
                                w2_sb[:, kj, mi * P:(mi + 1) * P],
                                z2_sb[:, kj, cs],
                                start=(kj == 0),
                                stop=(kj == HT - 1),
                            )
                        nc.vector.tensor_mul(
                            s_sb[:, mi, cs], pss[u][:], d1_sb[:, mi, cs]
                        )
                    return
                for kj in range(HT):
                    for u, mi in enumerate(range(g * 4, g * 4 + 4)):
                        nc.tensor.matmul(
                            pss[u][:],
                            w2_sb[:, kj, mi * P:(mi + 1) * P],
                            z2_sb[:, kj, cs],
                            start=(kj == 0),
                            stop=(kj == HT - 1),
                        )
                for u, mi in enumerate(range(g * 4, g * 4 + 4)):
                    nc.vector.tensor_mul(
                        s_sb[:, mi, cs], pss[u][:], d1_sb[:, mi, cs]
                    )

            def dydx_block(c):
                cs = slice(c * CH, (c + 1) * CH)
                psd = psum(f"dydx{c}", rows=DX)
                for ki in range(HT):
                    nc.tensor.matmul(
                        psd[:], w1r_sb[:, ki * DX:(ki + 1) * DX], s_sb[:, ki, cs],
                        start=(ki == 0), stop=(ki == HT - 1),
                    )
                nc.vector.tensor_copy(dydx_sb[:, cs], psd[:])
                nc.sync.dma_start(out=t["dydx"].ap()[:, cs], in_=dydx_sb[:, cs])

            # ---- emission order (PE stream order) -----------------------
            # y/d2w blocks trail their producer by one PE phase so the PE
            # never waits on the ACT tanh/Square drain chain.
            layer1(0, 0)
            layer1(0, 1)
            layer1(1, 0)
            fwd_group(0, 0)   # starts early, paced by layer-1 tanh + W2T DMA
            layer1(1, 1)
            fwd_group(0, 1)
            d1_block(0)
            d1_block(1)
            fwd_group(1, 0)
            y_block(0)
            d2w_block(0)
            fwd_group(1, 1)

            s_sb = w2tp.tile([P, HT, NXL], F32R, tag="w2t_s")  # reuses w2t slot
            bwd_group(0, 0)
            y_block(1)
            d2w_block(1)
            nc.sync.dma_start(out=t["y"].ap(), in_=y_sb[:])
            bwd_group(0, 1)
            bwd_group(1, 0, chain=True)
            dydx_block(0)
            bwd_group(1, 1, chain=True)
            dydx_block(1)


def _build():
    nc = bacc.Bacc("TRN2", target_bir_lowering=False, debug=False, num_devices=NCORES)
    t = {}
    for name, shape, dt in [
        ("xT", [P, NXL], F32R),
        ("w1t", [P, 2 * P], F32R),
        ("w1r", [P, HT * DX], F32R),
        ("w2", [H, H], F32R),
        ("w2t", [H, H], F32R),
        ("b1r", [P, HT], F32),
        ("b2r", [P, HT], F32),
        ("w3mm", [P, HT], F32R),
        ("w3p", [P, HT], F32),
        ("w3n", [P, HT], F32),
        ("b3", [1, 1], F32),
    ]:
        t[name] = nc.dram_tensor(name, shape, dt, kind="ExternalInput")
    t["y"] = nc.dram_tensor("y", [DO, NXL], F32, kind="ExternalOutput")
    t["dydx"] = nc.dram_tensor("dydx", [DX, NXL], F32, kind="ExternalOutput")
    _emit(nc, t)
    nc.compile()
    return nc


_NC = None


def _get_nc():
    global _NC
    if _NC is None:
        _NC = _build()
    return _NC


def _prep_in_maps(inputs):
    x = np.asarray(inputs["x"], dtype=np.float32)
    W1 = np.asarray(inputs["W1"], dtype=np.float32)
    b1 = np.asarray(inputs["b1"], dtype=np.float32)
    W2 = np.asarray(inputs["W2"], dtype=np.float32)
    b2 = np.asarray(inputs["b2"], dtype=np.float32)
    W3 = np.asarray(inputs["W3"], dtype=np.float32)
    b3 = np.asarray(inputs["b3"], dtype=np.float32)

    # xT replicated to partition offsets 0/32/64/96 for 4-wide packed layer-1
    xT = _round_tf32(x.T)  # [8, 8192]
    xT_q = np.zeros((P, NX), dtype=np.float32)
    for g in range(4):
        xT_q[32 * g:32 * g + DX] = xT
    # w1t packed per row-group: block (g, half) holds W1T[:, (half*4+g)-th tile]
    w1t_q = np.zeros((P, 2 * P), dtype=np.float32)
    W1T_r = _round_tf32(W1.T)  # [8, 1024]
    for half in range(2):
        for g in range(4):
            hi = half * 4 + g
            w1t_q[32 * g:32 * g + DX, half * P:(half + 1) * P] = W1T_r[:, hi * P:(hi + 1) * P]
    # W1 row-tiles stacked along free dim: [128, 8*8], col block ki = W1[ki*128+p, :]
    w1r = _round_tf32(W1.reshape(HT, P, DX).transpose(1, 0, 2).reshape(P, HT * DX))
    w2 = _round_tf32(W2)
    w2t = _round_tf32(W2.T)
    b1r = np.ascontiguousarray(b1.reshape(HT, P).T)  # [128, 8]
    b2r = np.ascontiguousarray(b2.reshape(HT, P).T)
    w3r = np.ascontiguousarray(W3[0].reshape(HT, P).T)  # [128, 8]
    w3mm = _round_tf32(w3r)
    w3n = -w3r
    b3v = b3.reshape(1, 1)

    shared = {
        "w1t": w1t_q, "w1r": w1r, "w2": w2, "w2t": w2t,
        "b1r": b1r, "b2r": b2r, "w3mm": w3mm, "w3p": w3r, "w3n": w3n, "b3": b3v,
    }
    in_maps = []
    for c in range(NCORES):
        m = dict(shared)
        m["xT"] = np.ascontiguousarray(xT_q[:, c * NXL:(c + 1) * NXL])
        in_maps.append(m)
    return in_maps


def run(inputs, trace=False):
    nc = _get_nc()
    in_maps = _prep_in_maps(inputs)
    res = run_bass_kernel_spmd(nc, in_maps, list(range(NCORES)), trace=trace)
    y = np.concatenate([res.results[c]["y"][0] for c in range(NCORES)])[:, None]
    dydx = np.concatenate([res.results[c]["dydx"] for c in range(NCORES)], axis=1)[:, :, None]
    out = (y.astype(np.float32), dydx.astype(np.float32))
    return out, res.exec_time_ns


def kernel(**inputs):
    out, _ = run(inputs, trace=False)
    return out
